# revision 1
# baseline (speedup 1.0000x reference)
"""Trainium2 Bass kernel for a 2-step BasicNCA2D cell update (fp8 DoubleRow).

Strategy
--------
Data-parallel over batch: 8 images, one per NeuronCore. Per core the two NCA
steps are fused on-chip (x never round-trips to DRAM between steps).

Per step the math is
    y  = depthwise_conv5x5(x, conv_w) + conv_b        (reflect padding)
    h  = relu([x, y] @ fc0_w + fc0_b)
    dx = h @ fc1_w
    x' = concat([x[..., :1], x[..., 1:] + dx])

conv+fc0 are fused into accumulating matmuls over shifted 4-row x blocks:
    h_pre = sum_{di,dj} x_shift(di,dj) @ M[di,dj],
    M[di,dj] = diag(conv_w[di,dj]) @ fc0_w[24:] (+ fc0_w[:24] at center)

The conv-path operands (x blocks and the fused M matrices) are quantized to
fp8 e4m3 (weights pre-scaled by 512 to dodge subnormals; the 1/512 rides the
relu's activation scale). This enables MatmulPerfMode.DoubleRow: the two
4-row input blocks feeding one output group become the two K-slabs of a
single matmul at 0.5 cycles/row, so conv+fc0 for 4 rows x 512 cols is 5
DoubleRow matmuls (one per horizontal tap) instead of 10 fp32r ones -- 4x
less PE time. h and fc1 stay bf16 (accuracy), the residual stream stays
bf16/f32 (DVE adds), giving ~1.3e-2 rel err vs the 2e-2 gate.

Layout: x blocks live inside one SBUF tensor per stage so the two K-slabs
of each DoubleRow matmul are adjacent slots in one AP. Stage 0 uses a
17-slot ring (slot 16 = DMA'd duplicate of slot 0 so pair (15,16) never
wraps); stage 1 is a linear 129-slot array (each block written once, no
WAR). Stage-0 blocks (reflect halos baked in) are pre-quantized host-side
and DMA'd 4 per descriptor set; stage-1 blocks are produced on-chip by
partition-shifted bf16->fp8 copies (mostly Pool, ~1/4 on DVE/ACT to smooth
engine load) from the stage-0 residual output, with Pool patching reflect
halo cols via strided 2-point APs batched over 4 slots, plus top/bottom
reflect rows. Engine split per 4-row group: PE 5xDR-conv + 1x bf16 fc1,
ScalarE relu (scale+bias fold the 1/512), DVE residual add, Pool converts/
halos. All four PSUM tiles share two 4-deep rotations (8 banks) so the PE
can run several groups ahead of the relu/add drains; stages are offset by
14 groups in the software pipeline. I/O is bf16.

Cost model 220.8us/pass vs 667.5us for the fp32r baseline (PE busy 192us:
conv 137us at 94% of DoubleRow capacity + fc1 55us). HW rel err 1.30e-2
(budget 2e-2), dominated by e4m3 quantization of x and the fused weights.
"""

import numpy as np
import ml_dtypes

import concourse.mybir as mybir
import concourse.tile as tile
from concourse import bacc
from concourse.bass_utils import run_bass_kernel_spmd

F32 = mybir.dt.float32
F8 = mybir.dt.float8e4
BF16 = mybir.dt.bfloat16
DR = mybir.MatmulPerfMode.DoubleRow

H = 512
W = 512
C = 24
CP = 32
HD = 32
NCORES = 8
NGRP = H // 4          # 128 output groups of 4 rows
NBLK = NGRP + 1        # 129 shifted input blocks per stage
NBATCH = NBLK // 4     # 32 full 4-block DMA batches (+1 tail block)
RP = 520               # ring slot pitch (516 used)
SCALE = 512.0


def _build_nc(steps: int, repeat: int = 1):
    nc = bacc.Bacc("TRN2", target_bir_lowering=False, debug=False)

    X8D = nc.dram_tensor("X8D", [NBATCH + 1, 4, CP, 4, RP], F8, kind="ExternalInput")
    XHID = nc.dram_tensor("XHID", [NBATCH, 4, CP, 4, W], BF16, kind="ExternalInput")
    WAB8 = nc.dram_tensor("WAB8", [128, 5, 2, 128], F8, kind="ExternalInput")
    WCB = nc.dram_tensor("WCB", [128, 128], BF16, kind="ExternalInput")
    BIAS = nc.dram_tensor("BIAS", [128, 1], F32, kind="ExternalInput")
    YD = nc.dram_tensor("YD", [NBATCH, 4, CP, 4, W], BF16, kind="ExternalOutput")

    with tile.TileContext(nc) as tc:
        with (
            tc.tile_pool(name="wpool", bufs=1) as wpool,
            tc.tile_pool(name="rpool", bufs=1) as rpool,
            tc.tile_pool(name="xpool", bufs=1) as xpool,
            tc.tile_pool(name="hpool", bufs=4) as hpool,
            tc.tile_pool(name="pp", bufs=4, space="PSUM") as pp,
            tc.tile_pool(name="ppdx", bufs=4, space="PSUM") as ppdx,
        ):
            wab_t = wpool.tile([128, 5, 2, 128], F8, tag="wab")
            nc.sync.dma_start(wab_t[:], WAB8.ap())
            wc_t = wpool.tile([128, 128], BF16, tag="wc")
            nc.sync.dma_start(wc_t[:], WCB.ap())
            bias_t = wpool.tile([128, 1], F32, tag="bias")
            nc.sync.dma_start(bias_t[:], BIAS.ap())

            # stage-0: 17-slot fp8 ring (slot16 = DMA'd dup of slot0);
            # stages >=1: linear (one slot per block, pairs always adjacent)
            rings = [
                rpool.tile(
                    [128, 17 if s == 0 else NBLK, RP], F8,
                    tag=f"r{s}", name=f"r{s}",
                )
                for s in range(steps)
            ]
            # per-stage residual-stream rings (x after s steps), 4-row aligned
            xhib = [
                xpool.tile([128, 16, W], BF16, tag=f"xhi{s}", name=f"xhi{s}")
                for s in range(steps)
            ]
            outb = xpool.tile([128, 16, W], BF16, tag="outb")

            pend = [dict() for _ in range(steps)]
            cvt_idx = [0]

            CVT_CYCLE = "PPPDPPPA"

            def _convert(dst, src):
                eng = CVT_CYCLE[cvt_idx[0] % len(CVT_CYCLE)]
                cvt_idx[0] += 1
                if eng == "P":
                    nc.gpsimd.tensor_copy(dst, src)
                elif eng == "D":
                    nc.vector.tensor_copy(dst, src)
                else:
                    nc.scalar.activation(
                        dst, src, mybir.ActivationFunctionType.Copy
                    )

            def load_batch(j):
                """Stage-0 DMA loads: X8 blocks 4j..4j+3 (+dups) and XHI."""
                if j > NBATCH:
                    return
                if j < NBATCH:
                    s0 = (4 * j) % 16
                    nc.sync.dma_start(
                        rings[0][:, s0 : s0 + 4, :], X8D.ap()[j]
                    )
                    nc.sync.dma_start(
                        xhib[0][:, s0 : s0 + 4, :], XHID.ap()[j]
                    )
                if j > 0 and j % 4 == 0:
                    # block 4j lands in slot 0; duplicate it into slot 16
                    nc.sync.dma_start(
                        rings[0][:, 16, :], X8D.ap()[j, :, :, 0, :]
                    )

            def part1(s, g):
                """conv+fc0 (5 DoubleRow fp8 matmuls) + relu for group g."""
                ring = rings[s]
                sg = g % 16 if s == 0 else g
                hp = pp.tile([128, W], F32, tag="hp", name=f"hp{s}_{g}")
                for dj in range(5):
                    nc.tensor.matmul(
                        hp[:],
                        wab_t[:, dj],
                        ring[:, sg : sg + 2, dj : dj + W],
                        start=(dj == 0),
                        stop=(dj == 4),
                        perf_mode=DR,
                    )
                h = hpool.tile([128, W], BF16, tag=f"h{s}", name=f"h{s}_{g}")
                nc.scalar.activation(
                    h[:], hp[:], mybir.ActivationFunctionType.Relu,
                    bias=bias_t[:], scale=1.0 / SCALE,
                )
                pend[s][g] = h

            def part2(s, g):
                """fc1 + residual add (+ next-stage fp8 block production)."""
                last = s == steps - 1
                h = pend[s].pop(g)
                dxp = ppdx.tile([128, W], F32, tag="dx", name=f"dx{s}_{g}")
                nc.tensor.matmul(dxp[:], wc_t[:], h[:], start=True, stop=True)

                sg16 = g % 16
                if last:
                    nc.vector.tensor_add(
                        outb[:, sg16, :], dxp[:], xhib[s][:, sg16, :]
                    )
                    if g >= NGRP - 4:
                        # last batch: per-block DMAs to shorten the drain
                        nc.sync.dma_start(
                            YD.ap()[g // 4, :, :, g % 4, :],
                            outb[:, g % 16, :],
                        )
                    elif g % 4 == 3:
                        s0 = (g - 3) % 16
                        nc.sync.dma_start(
                            YD.ap()[g // 4], outb[:, s0 : s0 + 4, :]
                        )
                    return

                nc.vector.tensor_add(
                    xhib[s + 1][:, sg16, :], dxp[:], xhib[s][:, sg16, :]
                )
                # next-stage fp8 block k = g (rows 4k-2..4k+1), shifted halves
                r = rings[s + 1]
                xh = xhib[s + 1]
                k = g
                if k >= 1:
                    _convert(r[0:64, k, 2 : 2 + W], xh[64:128, (k - 1) % 16, :])
                _convert(r[64:128, k, 2 : 2 + W], xh[0:64, k % 16, :])

                def halo_cols_batch(s0, nslot):
                    # reflect halo cols for slots s0..s0+nslot-1, two strided
                    # copies: cols (0,514)<-(4,512) and (1,515)<-(3,511)
                    src = r[:, s0 : s0 + nslot, :]
                    for vc, pc in ((0, 4), (1, 3)):
                        nc.gpsimd.tensor_copy(
                            src[:, :, vc : vc + 515 : 514],
                            src[:, :, pc : pc + 509 : 508],
                        )

                if k % 4 == 3:
                    # blocks k-3..k fully converted: batch their halo cols
                    halo_cols_batch(k - 3, 4)
                    if k == 3:
                        # block 0 reflect rows: row -2 := row 2 (block 1's
                        # partitions 0:32), row -1 := row 1 (own 96:128)
                        nc.gpsimd.tensor_copy(
                            r[0:32, 0, 0:516], r[0:32, 1, 0:516]
                        )
                        nc.gpsimd.tensor_copy(
                            r[32:64, 0, 0:516], r[96:128, 0, 0:516]
                        )
                if k == NGRP - 1:
                    # block 128: rows 510,511 real; 512:=510, 513:=509
                    kb = NBLK - 1
                    nc.gpsimd.tensor_copy(
                        r[0:64, kb, 2 : 2 + W], xh[64:128, k % 16, :]
                    )
                    nc.gpsimd.tensor_copy(
                        r[64:96, kb, 0:516], r[0:32, kb, 0:516]
                    )
                    nc.gpsimd.tensor_copy(
                        r[96:128, kb, 0:516], r[96:128, kb - 1, 0:516]
                    )
                    halo_cols_batch(kb, 1)

            for _rep in range(repeat):
                for st in pend:
                    st.clear()
                load_batch(0)
                load_batch(1)
                p1n = [0] * steps
                p2n = [0] * steps
                i = 0
                while p2n[steps - 1] < NGRP and i < 400:
                    if i % 4 == 0:
                        load_batch(i // 4 + 2)
                    for s in range(steps):
                        # once the prior stage is drained, catch up 2/iter
                        drained = s > 0 and p2n[s - 1] >= NGRP
                        budget = 2 if drained else 1
                        for _ in range(budget):
                            if p1n[s] < NGRP and (
                                p1n[s] <= i - 14 * s or drained
                            ):
                                part1(s, p1n[s])
                                p1n[s] += 1
                            if p2n[s] < NGRP and (
                                p2n[s] <= p1n[s] - 3 or p1n[s] >= NGRP
                            ):
                                part2(s, p2n[s])
                                p2n[s] += 1
                    i += 1

    nc.compile()
    return nc


_NC_CACHE = {}
_REPEAT = 1


def _get_nc(steps):
    key = (steps, _REPEAT)
    if key not in _NC_CACHE:
        _NC_CACHE[key] = _build_nc(steps, repeat=_REPEAT)
    return _NC_CACHE[key]


def _q8(a):
    return np.clip(a, -240, 240).astype(ml_dtypes.float8_e4m3)


def _prep_weights(conv_w, conv_b, fc0_w, fc0_b, fc1_w):
    conv_w = np.asarray(conv_w, np.float64)[:, :, 0, :]  # [5,5,24]
    W1 = np.asarray(fc0_w, np.float64)[:C]  # [24,32]
    W2 = np.asarray(fc0_w, np.float64)[C:]  # [24,32]
    fc1_w = np.asarray(fc1_w, np.float64)  # [32,23]

    # M[ki, kj] = diag(conv_w[ki,kj]) @ W2 (+ W1 at center)
    M = conv_w[:, :, :, None] * W2[None, None, :, :]  # [5,5,24,32]
    M[2, 2] += W1

    WAB = np.zeros((2, 5, 128, 128), np.float64)
    for dj in range(5):
        for r in range(4):
            for f in range(4):
                ka = r - f  # di+2 for slab A (block g)
                if 0 <= ka <= 4:
                    WAB[0, dj, r * 32 : r * 32 + C, f * 32 : f * 32 + HD] = M[ka, dj]
                kb = r + 4 - f  # di+2 for slab B (block g+1)
                if 0 <= kb <= 4:
                    WAB[1, dj, r * 32 : r * 32 + C, f * 32 : f * 32 + HD] = M[kb, dj]

    # WAB8[k, dj, slab, m] = q8(512 * WAB[slab, dj, k, m])
    WAB8 = _q8(SCALE * WAB.transpose(2, 1, 0, 3))

    WC = np.zeros((128, 128), np.float64)
    for f in range(4):
        WC[f * 32 : f * 32 + HD, f * 32 + 1 : f * 32 + C] = fc1_w
    WCB = WC.astype(ml_dtypes.bfloat16)

    bias_eff = (
        np.asarray(fc0_b, np.float64) + np.asarray(conv_b, np.float64) @ W2
    ).astype(np.float32)
    BIAS = np.tile(bias_eff, 4).reshape(128, 1)
    return WAB8, WCB, BIAS


def _prep_image(x_chw):
    """x_chw [C,H,W] f32 -> (X8D fp8 batched blocks, XHID bf16 batched)."""
    xp = np.zeros((CP, H + 4, W + 4), np.float32)
    xp[:C] = np.pad(x_chw, ((0, 0), (2, 2), (2, 2)), mode="reflect")
    x8 = _q8(xp)  # [32, 516, 516]

    X8D = np.zeros((NBATCH + 1, 4, CP, 4, RP), ml_dtypes.float8_e4m3)
    # block k = 4j+s covers padded rows 4k..4k+3 (= image rows 4k-2..4k+1)
    rows = x8[:, : 4 * NBLK, :]  # [32, 516, 516]
    blk = rows.reshape(CP, NBLK, 4, W + 4).transpose(1, 2, 0, 3)  # [129,4,32,516]
    X8D[: NBATCH, :, :, :, : W + 4] = (
        blk[: 4 * NBATCH].reshape(NBATCH, 4, 4, CP, W + 4).transpose(0, 2, 3, 1, 4)
    )
    X8D[NBATCH, :, :, 0, : W + 4] = blk[4 * NBATCH]

    x32 = np.zeros((CP, H, W), np.float32)
    x32[:C] = x_chw
    XHID = (
        x32.reshape(CP, NBATCH, 4, 4, W)
        .transpose(1, 3, 0, 2, 4)
        .astype(ml_dtypes.bfloat16)
    )  # [32, 4, 32, 4, 512]: [j, r, c, s, :]
    return X8D, XHID


def _run_pass(x_chw, WAB8, WCB, BIAS, steps):
    """One device invocation: `steps` NCA steps on x [B, C, H, W] f32."""
    B = x_chw.shape[0]
    nc = _get_nc(steps)
    in_maps = []
    for i in range(NCORES):
        X8D, XHID = _prep_image(x_chw[i % B])
        in_maps.append({"X8D": X8D, "XHID": XHID, "WAB8": WAB8, "WCB": WCB,
                        "BIAS": BIAS})
    res = run_bass_kernel_spmd(nc, in_maps, core_ids=list(range(NCORES)))
    globals()["LAST_RESULTS"] = res
    out = np.empty((B, C, H, W), np.float32)
    for i in range(B):
        yd = np.asarray(res.results[i]["YD"])  # [32, 4, 32, 4, 512] bf16
        y32 = yd.transpose(2, 0, 3, 1, 4).reshape(CP, H, W)
        out[i] = y32[:C].astype(np.float32)
    return out


def kernel(x, conv_w, conv_b, fc0_w, fc0_b, fc1_w, steps):
    steps = int(steps)
    x = np.asarray(x, np.float32)
    B = x.shape[0]
    assert x.shape == (B, H, W, C) and 1 <= B <= NCORES, x.shape
    if steps <= 0:
        return x.copy()

    WAB8, WCB, BIAS = _prep_weights(conv_w, conv_b, fc0_w, fc0_b, fc1_w)
    x_chw = np.ascontiguousarray(x.transpose(0, 3, 1, 2))
    # device pipeline supports 2 fused steps; decompose larger step counts
    while steps > 0:
        n = 2 if steps >= 2 else 1
        x_chw = _run_pass(x_chw, WAB8, WCB, BIAS, n)
        steps -= n
    return np.ascontiguousarray(x_chw.transpose(0, 2, 3, 1)).astype(np.float32)


if __name__ == "__main__":
    rng = np.random.default_rng(0)
    inputs = {
        "x": rng.standard_normal((8, H, W, C), dtype=np.float32),
        "conv_w": (rng.standard_normal((5, 5, 1, C)) * 0.1).astype(np.float32),
        "conv_b": (rng.standard_normal((C,)) * 0.1).astype(np.float32),
        "fc0_w": (rng.standard_normal((2 * C, HD)) * 0.1).astype(np.float32),
        "fc0_b": (rng.standard_normal((HD,)) * 0.1).astype(np.float32),
        "fc1_w": (rng.standard_normal((HD, C - 1)) * 0.1).astype(np.float32),
        "steps": 2,
    }
    out = kernel(**inputs)
    print(out.shape, out.dtype)

